# revision 1
# baseline (speedup 1.0000x reference)
"""GATv2 dense-attention kernel for Trainium2 — v2 (schedule-optimized).

Math per batch b, head h (W=128 nodes, F=64 in-feats, OUTF=64, H=2):
  fsrc = x @ w_src.T + b_src           # [W, H*OUTF]
  fdst = x @ w_dst.T + b_dst
  e[i,j,h]  = sum_f a[h,f] * leakyrelu(fsrc[j,h,f] + fdst[i,h,f], 0.2)
  alpha     = softmax_j(e)
  out[i,f]  = mean_h sum_j alpha[i,j,h] * fsrc[j,h,f]

Device decomposition (same math as v1):
  leakyrelu(z) = 0.2*z + 0.8*relu(z); the fdst rank-1 term cancels in
  softmax_j and is dropped; e = 0.8*(a . relu(z)) + 0.2*sS_h[j], with the
  relu part computed as one [hf=128, j=128] tensor-scalar op per i and the
  a-contraction as a per-i PE matmul with stationary bf16 qhat columns.

v2 scheduling structure:
  - x is pre-transposed ON HOST (bf16, ones row appended) and packed with
    the weights into two DMAs; no on-device transpose.
  - ALL NB per-batch projection stages run upfront (proj pool holds NB
    generations) so the z phase has no cross-batch head dependencies.
  - per-i relu ops split DVE/Pool/Act = 20/6/6 per 32, interleaved so each
    engine's queue drains at the rate PE consumes the merged stream.
  - ss columns accumulate into one persistent PSUM tile (no ring WAR).
  - tails are emitted inside the next batch's z stream; the last batch's
    exp is split into i-halves so the drain overlaps the z tail.
"""

import functools
import sys

sys.path.insert(0, "/opt/trn_rl_repo")

import numpy as np

import bass_rust
import concourse.bass as bass
import concourse.mybir as mybir
import concourse.tile as tile
from concourse.bass_utils import run_bass_kernel_spmd

B, W, F = 64, 128, 64
H, OUTF = 2, 64
HO = H * OUTF  # 128
NCORES = 8
NB = B // NCORES  # batches per core
FP32 = mybir.dt.float32
BF16 = mybir.dt.bfloat16

CW = 3 * HO + 4          # cpack columns: wsrc|wdst|wsrch|acols|acol2
TAIL_AT = 20


def _z_pattern(nd, np_, na, order="DPA"):
    """Weighted round-robin interleave of engine assignments over W slots."""
    w = dict(D=nd, P=np_, A=na)
    issued = dict(D=0, P=0, A=0)
    out = []
    n = nd + np_ + na
    for k in range(n):
        e = max(order, key=lambda x: w[x] * (k + 1) / n - issued[x])
        issued[e] += 1
        out.append(e)
    return out


# Per-batch z-op split (engine-balance LP optimum).  Pool owns the upfront
# head copies, so the first two batches lean on DVE/Act while Pool drains
# its copy backlog; later batches give Pool extra ops to compensate.
# Pool (gpsimd) cannot touch PSUM on HW, so all PSUM->SBUF copies live on
# Act (DVE for the first two batches, during the DMA ramp); Pool gets the
# largest z share its 272.8ns/op rate supports.
_ZPAT = ([_z_pattern(86, 31, 11), _z_pattern(83, 30, 15)]
         + [_z_pattern(76, 29, 23) for _ in range(NB - 2)])

_wait_nop_counter = [0]
_WAIT_BUDGET = {}
_WAIT_BUDGET_DEFAULT = 1


def _legalize_waits(nc, nop_budget=1):
    """This container's walrus codegen rejects instructions carrying more than
    a struct-dependent number of sync waits (1 for Matmult S3_LW / Drain, 2
    for most compute structs).  Move excess semaphore waits onto same-engine
    NoOps inserted just before the offender."""
    for f in nc.m.functions:
        for blk in f.blocks:
            out = []
            changed = False
            for inst in blk.instructions:
                si = inst.sync_info
                if si is not None:
                    max_waits = _WAIT_BUDGET.get(str(inst.opcode), _WAIT_BUDGET_DEFAULT)
                    waits = list(si.on_wait)
                    movable = [w for w in waits
                               if w.sync_type == "semaphore"
                               and w.wait_mode == "sem-ge-imm"
                               and not w.wait_reg]
                    fixed = [w for w in waits if w not in movable]
                    budget = max(max_waits - len(fixed), 0)
                    if len(movable) > budget:
                        keep = movable[len(movable) - budget:] if budget else []
                        excess = movable[:len(movable) - budget] if budget else movable
                        for i in range(0, len(excess), nop_budget):
                            chunk = excess[i:i + nop_budget]
                            _wait_nop_counter[0] += 1
                            nop = bass_rust.InstNoOp(
                                name=f"legalize-wait-nop-{_wait_nop_counter[0]}",
                                ins=[], outs=[])
                            nop.engine = inst.engine
                            nop.sync_info = mybir.SyncInfo(on_wait=chunk, on_update=[])
                            out.append(nop)
                        inst.sync_info = mybir.SyncInfo(
                            on_wait=fixed + keep, on_update=list(si.on_update))
                        changed = True
                out.append(inst)
            if changed:
                blk.instructions = out


@functools.lru_cache(maxsize=1)
def _build():
    nc = bass.Bass("TRN2", target_bir_lowering=False)
    AF = mybir.ActivationFunctionType
    OP = mybir.AluOpType

    # cx: [cpack | xt for batches 0-1]; xt27: batches 2..NB-1.
    cx_d = nc.dram_tensor("cx", [HO, CW + 2 * W], BF16, kind="ExternalInput")
    xt27_d = nc.dram_tensor("xt27", [F + 1, (NB - 2) * W], BF16, kind="ExternalInput")
    out_d = nc.dram_tensor("out", [NB, W, OUTF], FP32, kind="ExternalOutput")

    with tile.TileContext(nc) as tc:
        with tc.tile_pool(name="const", bufs=1) as cpool, \
             tc.tile_pool(name="proj", bufs=NB) as proj, \
             tc.tile_pool(name="qhat", bufs=3) as qpool, \
             tc.tile_pool(name="soft", bufs=3) as soft, \
             tc.tile_pool(name="pps", bufs=3, space="PSUM") as pps, \
             tc.tile_pool(name="tps", bufs=2, space="PSUM") as tps, \
             tc.tile_pool(name="ssp", bufs=1, space="PSUM") as ssp, \
             tc.tile_pool(name="eps", bufs=2, space="PSUM") as eps:

            cx = cpool.tile([HO, CW + 2 * W], BF16)
            xt27 = cpool.tile([F + 1, (NB - 2) * W], BF16)
            ones = cpool.tile([W, 1], FP32)
            nc.sync.dma_start(cx[:], cx_d[:])
            nc.sync.dma_start(xt27[:], xt27_d[:])
            nc.vector.memset(ones[:], 1.0)

            wsrc = cx[0:F + 1, 0:HO]
            wdst = cx[0:F + 1, HO:2 * HO]
            wsrch = cx[0:F + 1, 2 * HO:3 * HO]
            acols = cx[:, 3 * HO:3 * HO + 2]      # 0.8*a block-diag
            acol2 = cx[:, 3 * HO + 2:3 * HO + 4]  # 0.2*a block-diag
            ssall = ssp.tile([W, 2 * NB], FP32)   # per-batch ss columns

            def xt_sl(b):
                if b < 2:
                    return cx[0:F + 1, CW + b * W:CW + (b + 1) * W]
                return xt27[:, (b - 2) * W:(b - 1) * W]

            state = {}

            def stage_head(b):
                xs = xt_sl(b)
                hp = pps.tile([HO, 3 * W], FP32, tag="ps")
                fsrcT_ps = hp[:, 0:W]
                fdstT_ps = hp[:, W:2 * W]
                fsrcN_ps = hp[:, 2 * W:3 * W]
                nc.tensor.matmul(fsrcT_ps, wsrc, xs, start=True, stop=True)
                nc.tensor.matmul(fdstT_ps, wdst, xs, start=True, stop=True)
                nc.tensor.matmul(fsrcN_ps, xs, wsrch, start=True, stop=True)
                fsrcT_bf = proj.tile([HO, W], BF16, tag="fsrcT_bf")
                fdstT = proj.tile([HO, W], FP32, tag="fdstT")
                fsrcN = proj.tile([W, HO], FP32, tag="fsrcN")
                if b < 2:
                    # engines are otherwise idle during the DMA/projection
                    # ramp: spread the first batches' copies to cut startup
                    nc.vector.tensor_copy(fsrcT_bf[:], fsrcT_ps)
                    nc.vector.tensor_copy(fdstT[:], fdstT_ps)
                    nc.scalar.copy(fsrcN[:], fsrcN_ps)
                else:
                    nc.scalar.copy(fsrcT_bf[:], fsrcT_ps)
                    nc.scalar.copy(fdstT[:], fdstT_ps)
                    nc.scalar.copy(fsrcN[:], fsrcN_ps)
                state[b] = dict(fsrcT_bf=fsrcT_bf, fdstT=fdstT, fsrcN=fsrcN)

            def stage_head_ss(b):
                st = state[b]
                ss_ps = ssall[:, 2 * b:2 * b + 2]
                nc.tensor.matmul(ss_ps, st["fsrcT_bf"][:], acol2,
                                 start=True, stop=True)
                ssc = proj.tile([W, 2], FP32, tag="ssc")
                nc.scalar.copy(ssc[:], ss_ps)
                st["ssc"] = ssc

            def emit_z(b, i):
                st = state[b]
                if i == 0:
                    st["ET"] = eps.tile([W, 2 * W], FP32, tag="ET", name="ET")
                    st["qbig"] = qpool.tile([HO, W * W], BF16, tag="qbig",
                                            name="qbig")
                ET, qbig = st["ET"], st["qbig"]
                fsrcT_bf, fdstT = st["fsrcT_bf"], st["fdstT"]
                qs = qbig[:, W * i:W * (i + 1)]
                eng = _ZPAT[b][i]
                if eng == "A":
                    nc.scalar.activation(qs, fsrcT_bf[:], AF.Relu,
                                         bias=fdstT[:, i:i + 1], scale=1.0)
                elif eng == "P":
                    nc.gpsimd.tensor_scalar(
                        out=qs, in0=fsrcT_bf[:],
                        scalar1=fdstT[:, i:i + 1], scalar2=0.0,
                        op0=OP.add, op1=OP.max)
                else:
                    nc.vector.tensor_scalar(
                        out=qs, in0=fsrcT_bf[:],
                        scalar1=fdstT[:, i:i + 1], scalar2=0.0,
                        op0=OP.add, op1=OP.max)
                nc.tensor.matmul(ET[:, 2 * i:2 * i + 2], qs, acols,
                                 start=True, stop=True)

            def pt_view(t, h):
                return t[:].rearrange("j (i h) -> j i h", h=2)[:, :, h]

            def tail_alloc(b):
                st = state[b]
                st["PT"] = soft.tile([W, 2 * W], FP32, tag="PT", name="PT")
                st["ts"] = tps.tile([W, 2 + 2 * OUTF], FP32, tag="tp", name="ts")
                st["rzc"] = soft.tile([W, 2], FP32, tag="rzc", name="rzc")
                st["f_sb"] = soft.tile([W, OUTF], FP32, tag="f_sb", name="f_sb")
                st["f_out"] = soft.tile([W, OUTF], FP32, tag="f_out", name="f_out")

            def tail_exp(b, lo, hi):
                st = state[b]
                for h in range(H):
                    nc.scalar.activation(
                        pt_view(st["PT"], h)[:, lo:hi],
                        pt_view(st["ET"], h)[:, lo:hi],
                        AF.Exp, bias=st["ssc"][:, h:h + 1], scale=1.0)

            def tail_mm(b, lo, hi):
                st = state[b]
                PT, ts, fsrcN = st["PT"], st["ts"], st["fsrcN"]
                zc_ps = ts[:, 0:2]
                for h in range(H):
                    nc.tensor.matmul(zc_ps[lo:hi, h:h + 1],
                                     pt_view(PT, h)[:, lo:hi],
                                     ones[:], start=True, stop=True)
                for h in range(H):
                    fps = ts[:, 2 + h * OUTF:2 + (h + 1) * OUTF]
                    nc.tensor.matmul(fps[lo:hi, :],
                                     pt_view(PT, h)[:, lo:hi],
                                     fsrcN[:, h * OUTF:(h + 1) * OUTF],
                                     start=True, stop=True)

            def tail_fin(b, lo, hi):
                st = state[b]
                ts, rzc, f_sb, f_out = st["ts"], st["rzc"], st["f_sb"], st["f_out"]
                zc_ps = ts[:, 0:2]
                f0_ps = ts[:, 2:2 + OUTF]
                f1_ps = ts[:, 2 + OUTF:2 + 2 * OUTF]
                nc.vector.reciprocal(rzc[lo:hi, :], zc_ps[lo:hi, :])
                nc.vector.tensor_scalar_mul(
                    f_sb[lo:hi, :], f0_ps[lo:hi, :], rzc[lo:hi, 0:1])
                nc.vector.scalar_tensor_tensor(
                    out=f_out[lo:hi, :], in0=f1_ps[lo:hi, :],
                    scalar=rzc[lo:hi, 1:2], in1=f_sb[lo:hi, :],
                    op0=OP.mult, op1=OP.add)
                nc.sync.dma_start(out_d[b, lo:hi], f_out[lo:hi, :])

            def stage_tail(b):
                tail_alloc(b)
                tail_exp(b, 0, W)
                tail_mm(b, 0, W)
                tail_fin(b, 0, W)
                del state[b]

            for b in range(NB):
                stage_head(b)
            for b in range(NB):
                stage_head_ss(b)
            # last-batch tail runs as two staggered half-chains so the first
            # half's exp/matmul/store overlap the final z ops.
            LB = NB - 1
            for b in range(NB):
                for i in range(W):
                    emit_z(b, i)
                    if i == TAIL_AT and b > 0:
                        stage_tail(b - 1)
                    if b == LB:
                        if i == 64:
                            tail_alloc(LB)
                            tail_exp(LB, 0, W // 2)
                        elif i == 84:
                            tail_mm(LB, 0, W // 2)
                        elif i == 100:
                            tail_fin(LB, 0, W // 2)
            tail_exp(LB, W // 2, W)
            tail_mm(LB, W // 2, W)
            tail_fin(LB, W // 2, W)

    _legalize_waits(nc)
    return nc


def _make_consts(w_src, b_src, w_dst, b_dst, attn_w):
    wsrc_ext = np.concatenate([w_src.T, b_src[None, :]], axis=0)
    wdst_ext = np.concatenate([w_dst.T, b_dst[None, :]], axis=0)
    wsrc_half = 0.5 * wsrc_ext
    cpack = np.zeros((HO, CW), np.float32)
    cpack[0:F + 1, 0:HO] = wsrc_ext
    cpack[0:F + 1, HO:2 * HO] = wdst_ext
    cpack[0:F + 1, 2 * HO:3 * HO] = wsrc_half
    cpack[0:OUTF, 3 * HO] = 0.8 * attn_w[0]
    cpack[OUTF:HO, 3 * HO + 1] = 0.8 * attn_w[1]
    cpack[0:OUTF, 3 * HO + 2] = 0.2 * attn_w[0]
    cpack[OUTF:HO, 3 * HO + 3] = 0.2 * attn_w[1]
    return cpack


def _make_xt(x_core):
    """[NB, W, F] fp32 -> [F+1, NB*W] fp32 with an all-ones bias row."""
    xt = x_core.transpose(0, 2, 1)                       # [NB, F, W]
    xe = np.concatenate(
        [xt, np.ones((NB, 1, W), np.float32)], axis=1)   # [NB, F+1, W]
    return xe.transpose(1, 0, 2).reshape(F + 1, NB * W)  # [F+1, NB*W]


def kernel(x, w_src, b_src, w_dst, b_dst, attn_w):
    import ml_dtypes
    x = np.asarray(x, dtype=np.float32)
    cpack = _make_consts(np.asarray(w_src, np.float32), np.asarray(b_src, np.float32),
                         np.asarray(w_dst, np.float32), np.asarray(b_dst, np.float32),
                         np.asarray(attn_w, np.float32))
    nc = _build()
    in_maps = []
    for c in range(NCORES):
        xt = _make_xt(x[c * NB:(c + 1) * NB])
        cx = np.zeros((HO, CW + 2 * W), np.float32)
        cx[:, :CW] = cpack
        cx[0:F + 1, CW:] = xt[:, :2 * W]
        in_maps.append({
            "cx": np.ascontiguousarray(cx.astype(ml_dtypes.bfloat16)),
            "xt27": np.ascontiguousarray(xt[:, 2 * W:].astype(ml_dtypes.bfloat16)),
        })
    res = run_bass_kernel_spmd(nc, in_maps, core_ids=list(range(NCORES)))
    out = np.concatenate([r["out"] for r in res.results], axis=0)
    return out.astype(np.float32)

